# revision 2
# baseline (speedup 1.0000x reference)
"""PointNet feature extractor on 8 Trainium2 NeuronCores (Bass/Tile), v2.

Problem: x (8, 16384, 3) -> 3x [conv1d(k=1) + sync-BN (+ReLU)] ->
global max-pool -> out (8, 1088, 16384) where rows 0:1024 are the
broadcast global feature and rows 1024:1088 are the (transposed) local
(layer-0) features.

Sharding: data-parallel over batch, 1 batch per core.  BN statistics
(per-channel sum / sum-of-squares) are AllReduced across the 8 cores.

v2 changes vs v1:
  * 3 AllReduces total (one per BN layer) instead of 11: layer-2 sumsq
    for all 8 channel blocks plus the h1 channel-sum ride in one
    [128, 9] payload.
  * Layer-2 matmuls (95% of FLOPs) run in bf16 (PSUM still f32).
  * xT shares its SBUF slab with u1 (dead after layer 0).
  * Channel sums come free out of activation accumulators
    (sum(h) via the ReLU pass, sum(u1) = W1 @ sum(h0)).
  * All gfeat broadcast writes are issued back-to-back after the last
    AllReduce, spread over 4 DMA queues.

Key algebraic facts used (as in v1):
  * conv biases b0/b1/b2 cancel inside training-mode BN.
  * BN scale is positive (gamma=1), so max_n BN(u) = BN(max_n u).
  * mean of h2 = W2 @ (AllReduce sum of h1) / N_total.
"""

import functools
import numpy as np

B = 8
N = 16384          # points per batch == points per core (1 batch / core)
NTOT = B * N       # BN statistics population size
EPS = 1e-5
NCORES = 8
NCHUNK = 2048      # PSUM evacuation chunk (4 banks)
T = N // NCHUNK    # 8 chunks


def _body(nc, tc, io):
    import concourse.bass as bass
    from concourse import mybir

    f32 = mybir.dt.float32
    bf16 = mybir.dt.bfloat16
    AF = mybir.ActivationFunctionType
    OP = mybir.AluOpType
    AX = mybir.AxisListType
    RG = [list(range(NCORES))]

    out = io["out"]

    with (
        tc.tile_pool(name="singles", bufs=1) as singles,
        tc.tile_pool(name="big", bufs=1) as big,
        tc.tile_pool(name="scr", bufs=2) as scr,
        tc.tile_pool(name="stat", bufs=1) as stat,
        tc.tile_pool(name="statl", bufs=2) as statl,
        tc.tile_pool(name="psum", bufs=2, space="PSUM") as psum,
        tc.tile_pool(name="dram", bufs=1, space="DRAM") as dram,
    ):
        # ---------------- load inputs ----------------
        # xT shares its SBUF slot with u1 (phase B) via the pool tag: xT is
        # dead once the layer-0 matmuls are done.
        sb_xT = big.tile([3, N], f32, tag="u1x")
        nc.gpsimd.dma_start(sb_xT[:], io["xT"])
        sb_w0t = singles.tile([3, 64], f32)
        nc.gpsimd.dma_start(sb_w0t[:], io["w0t"])
        sb_w1t = singles.tile([64, 128], f32)
        nc.gpsimd.dma_start(sb_w1t[:], io["w1t"])
        sb_w2t = singles.tile([128, 1024], bf16)
        nc.gpsimd.dma_start(sb_w2t[:], io["w2t"])
        sb_g0 = singles.tile([64, 2], f32)
        nc.gpsimd.dma_start(sb_g0[:], io["g0be0"])
        sb_g1 = singles.tile([128, 2], f32)
        nc.gpsimd.dma_start(sb_g1[:], io["g1be1"])
        sb_g2 = singles.tile([128, 8], f32)
        nc.gpsimd.dma_start(sb_g2[:], io["g2p"])
        sb_be2 = singles.tile([128, 8], f32)
        nc.gpsimd.dma_start(sb_be2[:], io["be2p"])

        sb_eps = singles.tile([128, 1], f32)
        nc.vector.memset(sb_eps[:], EPS)

        def allreduce(src_ap, p, f, tag):
            """AllReduce add of an SBUF [p, f] region; returns SBUF tile.

            The collective output must live in the Shared DRAM scratchpad —
            with a Local (or pool) output the exec unit crashes under this
            runtime (NRT_EXEC_UNIT_UNRECOVERABLE).
            """
            d_in = dram.tile([p, f], f32, tag=f"ar_in_{tag}")
            d_out = nc.dram_tensor(
                f"cc_out_{tag}", [p, f], f32, kind="Internal", addr_space="Shared"
            )
            nc.gpsimd.dma_start(d_in[:], src_ap)
            nc.gpsimd.collective_compute(
                "AllReduce",
                OP.add,
                replica_groups=RG,
                ins=[d_in[:].opt()],
                outs=[d_out.ap().opt()],
            )
            red = stat.tile([p, f], f32, tag=f"ar_red_{tag}")
            nc.gpsimd.dma_start(red[:], d_out.ap())
            return red

        def bn_affine(red, gbe, p, tag):
            """From AllReduced [p,2] (sum, sumsq) + gamma/beta [p,2] make
            (scale, shift) [p,2] tile: y = scale*u + shift."""
            w = stat.tile([p, 6], f32, tag=f"bnw_{tag}")
            # w0 = mean, w1 = E[u^2]
            nc.scalar.mul(w[:, 0:2], red[:, 0:2], 1.0 / NTOT)
            # w2 = mean^2 ; w3 = var = E[u^2] - mean^2
            nc.vector.tensor_mul(w[:, 2:3], w[:, 0:1], w[:, 0:1])
            nc.vector.tensor_sub(w[:, 3:4], w[:, 1:2], w[:, 2:3])
            # w4 = sqrt(var + eps)
            nc.scalar.activation(w[:, 4:5], w[:, 3:4], AF.Sqrt, bias=sb_eps[:p, 0:1])
            # w5 = rstd
            nc.vector.reciprocal(w[:, 5:6], w[:, 4:5])
            sc = stat.tile([p, 2], f32, tag=f"bnsc_{tag}")
            # scale = gamma * rstd
            nc.vector.tensor_mul(sc[:, 0:1], gbe[:, 0:1], w[:, 5:6])
            # shift = beta - mean * scale
            nc.vector.tensor_mul(sc[:, 1:2], w[:, 0:1], sc[:, 0:1])
            nc.vector.tensor_sub(sc[:, 1:2], gbe[:, 1:2], sc[:, 1:2])
            return sc

        # ---------------- layer 0 ----------------
        u0 = big.tile([64, N], f32)
        s0p = stat.tile([64, T], f32)
        q0p = stat.tile([64, T], f32)
        for t in range(T):
            pa = psum.tile([128, NCHUNK], f32, tag="mm")
            for k in range(4):
                nc.tensor.matmul(
                    pa[0:64, k * 512:(k + 1) * 512],
                    sb_w0t[:],
                    sb_xT[:, t * NCHUNK + k * 512: t * NCHUNK + (k + 1) * 512],
                )
            # evacuate + per-chunk sum (ACT), per-chunk sumsq (ACT)
            nc.scalar.activation(
                u0[:, t * NCHUNK:(t + 1) * NCHUNK], pa[0:64, :], AF.Copy,
                accum_out=s0p[:, t:t + 1],
            )
            sq = scr.tile([128, NCHUNK], f32, tag="sq")
            nc.scalar.activation(
                sq[0:64, :], pa[0:64, :], AF.Square,
                accum_out=q0p[:, t:t + 1],
            )
        ar0 = stat.tile([64, 2], f32)
        nc.vector.reduce_sum(ar0[:, 0:1], s0p[:], axis=AX.X)
        nc.vector.reduce_sum(ar0[:, 1:2], q0p[:], axis=AX.X)
        red0 = allreduce(ar0[:], 64, 2, "bn0")
        sc0 = bn_affine(red0, sb_g0, 64, "bn0")

        # normalize + relu in place: h0 = relu(scale*u0 + shift); the ReLU
        # pass also accumulates sum(h0) for layer-1's mean (s1 = W1 @ sh0).
        sh0p = stat.tile([64, 2], f32)
        for c in range(2):
            h = N // 2
            nc.scalar.activation(
                u0[:, c * h:(c + 1) * h], u0[:, c * h:(c + 1) * h],
                AF.Relu, bias=sc0[:, 1:2], scale=sc0[:, 0:1],
                accum_out=sh0p[:, c:c + 1],
            )
        # local features -> output rows 1024:1088 (early, overlaps the rest)
        nc.sync.dma_start(out[1024:1088, :], u0[:])

        # ---------------- layer 1 ----------------
        u1 = big.tile([128, N], f32, tag="u1x")
        q1p = stat.tile([128, T], f32)
        for t in range(T):
            pb = psum.tile([128, NCHUNK], f32, tag="mm")
            for k in range(4):
                nc.tensor.matmul(
                    pb[:, k * 512:(k + 1) * 512],
                    sb_w1t[:],
                    u0[:, t * NCHUNK + k * 512: t * NCHUNK + (k + 1) * 512],
                )
            # evacuate on DVE, sumsq on ACT
            nc.vector.tensor_copy(u1[:, t * NCHUNK:(t + 1) * NCHUNK], pb[:])
            sq = scr.tile([128, NCHUNK], f32, tag="sq")
            nc.scalar.activation(
                sq[:], pb[:], AF.Square, accum_out=q1p[:, t:t + 1],
            )
        # s1 = W1 @ sum(h0)
        sh0 = stat.tile([64, 1], f32)
        nc.vector.tensor_add(sh0[:], sh0p[:, 0:1], sh0p[:, 1:2])
        ps1 = psum.tile([128, 1], f32, tag="mm")
        nc.tensor.matmul(ps1[:], sb_w1t[:], sh0[:])
        ar1 = stat.tile([128, 2], f32)
        nc.scalar.copy(ar1[:, 0:1], ps1[:])
        nc.vector.reduce_sum(ar1[:, 1:2], q1p[:], axis=AX.X)
        red1 = allreduce(ar1[:], 128, 2, "bn1")
        sc1 = bn_affine(red1, sb_g1, 128, "bn1")

        # h1 = relu(scale*u1 + shift) in bf16; ReLU pass accumulates sum(h1)
        h1 = big.tile([128, N], bf16)
        sh1p = stat.tile([128, 2], f32)
        for c in range(2):
            h = N // 2
            nc.scalar.activation(
                h1[:, c * h:(c + 1) * h], u1[:, c * h:(c + 1) * h],
                AF.Relu, bias=sc1[:, 1:2], scale=sc1[:, 0:1],
                accum_out=sh1p[:, c:c + 1],
            )

        # ---------------- layer 2 stats + max pool ----------------
        mx8 = singles.tile([128, 8], f32)
        q2s = singles.tile([128, 9], f32)
        nc.vector.tensor_add(q2s[:, 8:9], sh1p[:, 0:1], sh1p[:, 1:2])
        for j in range(8):
            mxp = statl.tile([128, T], f32, tag="mxp")
            q2p = statl.tile([128, T], f32, tag="q2p")
            for t in range(T):
                pc = psum.tile([128, NCHUNK], f32, tag="mm")
                for k in range(4):
                    nc.tensor.matmul(
                        pc[:, k * 512:(k + 1) * 512],
                        sb_w2t[:, j * 128:(j + 1) * 128],
                        h1[:, t * NCHUNK + k * 512: t * NCHUNK + (k + 1) * 512],
                    )
                sq = scr.tile([128, NCHUNK], f32, tag="sq")
                nc.scalar.activation(
                    sq[:], pc[:], AF.Square, accum_out=q2p[:, t:t + 1],
                )
                nc.vector.reduce_max(mxp[:, t:t + 1], pc[:], axis=AX.X)
            nc.vector.reduce_sum(q2s[:, j:j + 1], q2p[:], axis=AX.X)
            nc.vector.reduce_max(mx8[:, j:j + 1], mxp[:], axis=AX.X)

        # single AllReduce for all of layer-2: q2 (8 cols) + sum(h1) (1 col)
        red3 = allreduce(q2s[:], 128, 9, "bn2")

        # mean2[:, j] = (W2 @ sum_h1)_block_j / NTOT
        sh1b = stat.tile([128, 1], bf16)
        nc.vector.tensor_copy(sh1b[:], red3[:, 8:9])
        m2 = stat.tile([128, 8], f32)
        for j in range(8):
            pm = psum.tile([128, 1], f32, tag="mm")
            nc.tensor.matmul(pm[:], sb_w2t[:, j * 128:(j + 1) * 128], sh1b[:])
            nc.scalar.mul(m2[:, j:j + 1], pm[:], 1.0 / NTOT)

        # var = E[h2^2] - mean^2; scale2 = g2*rstd; gfeat = scale2*max + (be2 - mean*scale2)
        w2s = stat.tile([128, 8 * 4], f32)
        e2 = w2s[:, 0:8]
        m2sq = w2s[:, 8:16]
        var = w2s[:, 16:24]
        rstd = w2s[:, 24:32]
        nc.scalar.mul(e2, red3[:, 0:8], 1.0 / NTOT)
        nc.vector.tensor_mul(m2sq, m2[:], m2[:])
        nc.vector.tensor_sub(var, e2, m2sq)
        nc.scalar.activation(var, var, AF.Sqrt, bias=sb_eps[:, 0:1])
        nc.vector.reciprocal(rstd, var)
        sc2 = stat.tile([128, 8], f32)
        sh2 = stat.tile([128, 8], f32)
        nc.vector.tensor_mul(sc2[:], sb_g2[:], rstd)
        nc.vector.tensor_mul(sh2[:], m2[:], sc2[:])
        nc.vector.tensor_sub(sh2[:], sb_be2[:], sh2[:])
        gf = singles.tile([128, 8], f32)
        nc.vector.tensor_mul(gf[:], sc2[:], mx8[:])
        nc.vector.tensor_add(gf[:], gf[:], sh2[:])

        # broadcast gfeat along the free dim and write 64 MB of output,
        # spread across both HWDGE queues (SP + Activation)
        dma_engines = [nc.sync, nc.scalar]
        for j in range(8):
            bc = scr.tile([128, NCHUNK], f32, tag="sq")
            nc.vector.tensor_copy(bc[:], gf[:, j:j + 1].to_broadcast([128, NCHUNK]))
            src = bc[:].unsqueeze(1).broadcast_to([128, N // NCHUNK, NCHUNK])
            dma_engines[j % 2].dma_start(out[j * 128:(j + 1) * 128, :], src)


@functools.lru_cache(maxsize=1)
def build_program():
    import concourse.bacc as bacc
    import concourse.tile as tile
    from concourse import mybir

    f32 = mybir.dt.float32
    bf16 = mybir.dt.bfloat16
    nc = bacc.Bacc(
        "TRN2", target_bir_lowering=False, debug=False, num_devices=NCORES
    )
    io = {
        "xT": nc.dram_tensor("xT", [3, N], f32, kind="ExternalInput").ap(),
        "w0t": nc.dram_tensor("w0t", [3, 64], f32, kind="ExternalInput").ap(),
        "w1t": nc.dram_tensor("w1t", [64, 128], f32, kind="ExternalInput").ap(),
        "w2t": nc.dram_tensor("w2t", [128, 1024], bf16, kind="ExternalInput").ap(),
        "g0be0": nc.dram_tensor("g0be0", [64, 2], f32, kind="ExternalInput").ap(),
        "g1be1": nc.dram_tensor("g1be1", [128, 2], f32, kind="ExternalInput").ap(),
        "g2p": nc.dram_tensor("g2p", [128, 8], f32, kind="ExternalInput").ap(),
        "be2p": nc.dram_tensor("be2p", [128, 8], f32, kind="ExternalInput").ap(),
        "out": nc.dram_tensor("out", [1088, N], f32, kind="ExternalOutput").ap(),
    }
    with tile.TileContext(nc) as tc:
        _body(nc, tc, io)
    nc.compile()
    return nc


def make_in_maps(x, W0, W1, W2, g0, be0, g1, be1, g2, be2):
    x = np.asarray(x, np.float32)
    shared = {
        "w0t": np.ascontiguousarray(np.asarray(W0, np.float32).T),
        "w1t": np.ascontiguousarray(np.asarray(W1, np.float32).T),
        "w2t": np.ascontiguousarray(_to_bf16(np.asarray(W2, np.float32).T)),
        "g0be0": np.ascontiguousarray(
            np.stack([np.asarray(g0, np.float32), np.asarray(be0, np.float32)], 1)),
        "g1be1": np.ascontiguousarray(
            np.stack([np.asarray(g1, np.float32), np.asarray(be1, np.float32)], 1)),
        "g2p": np.ascontiguousarray(np.asarray(g2, np.float32).reshape(8, 128).T),
        "be2p": np.ascontiguousarray(np.asarray(be2, np.float32).reshape(8, 128).T),
    }
    return [
        {"xT": np.ascontiguousarray(x[i].T), **shared} for i in range(NCORES)
    ]


def _to_bf16(a):
    """float32 -> bfloat16 (round-to-nearest-even) as ml_dtypes array."""
    import ml_dtypes
    return a.astype(ml_dtypes.bfloat16)


def kernel(x, W0, b0, g0, be0, W1, b1, g1, be1, W2, b2, g2, be2):
    """Full inputs in, full output out.  b0/b1/b2 cancel inside BN."""
    from concourse.bass_utils import run_bass_kernel_spmd

    nc = build_program()
    in_maps = make_in_maps(x, W0, W1, W2, g0, be0, g1, be1, g2, be2)
    res = run_bass_kernel_spmd(nc, in_maps, core_ids=list(range(NCORES)))
    return np.stack([res.results[i]["out"] for i in range(NCORES)], axis=0)
